# revision 25
# baseline (speedup 1.0000x reference)
"""Trainium2 Bass kernel: batch-sharded fused attention (nn_Attention_48893907698232).

Reference computation (per batch b):
    q = x @ wq.T + bq ; k = x @ wk.T + bk ; v = x @ wv.T + bv
    q, k <- fixed RoPE rotation (same rotation at every position)
    out  = softmax((q @ k.T) / sqrt(512)) @ v

Strategy:
  * Data-parallel over batch: 16 batches -> 2 per NeuronCore, 8 cores, no
    collectives (attention is per-batch independent).
  * The RoPE rotation is position-independent here, so it is folded into the
    projection weights/biases on the host (w_rot = R @ w, b_rot = R @ b).
  * Host pre-transposes/pre-tiles x and weights into SBUF tile layout so every
    matmul contraction dim lands on SBUF partitions with zero on-chip
    transposes and every DMA row is contiguous:
      qT[o,s]  = sum_i wqT[i,o] * xT[i,s]     (PSUM out: o on partitions)
      kT[o,s]  likewise
      v[s,o]   = sum_i xT[i,s] * wvT[i,o]     (PSUM out: s on partitions)
      ST[sk,sq]= sum_d kT[d,sk] * qT[d,sq]    -> exp(scale*ST) on ACT -> ET
      out[sq,:]= sum_sk ET[sk,sq] * [1|v]     (ones column 0 => softmax rowsum
                                               lands in output column 0)
      out  <- out[:, 1:257] * (1 / out[:, 0])
  * Softmax runs without max-subtraction: scores are ~N(0, 0.75) for this
    problem so exp() is far from overflow.
  * Matmul operands are bf16 (full-rate TensorEngine, fp32 PSUM accumulate).
  * Startup is latency-tuned: a dependency-free junk-matmul warmup block
    (reads an uninitialized scratch tensor; PSUM discarded) dispatches at
    kernel entry so the PE clock ramp (0.65->1.2->2.4GHz over ~5us of
    CONTINUOUS execution; an idle gap freezes it) burns while the input
    DMAs land.  Input loads are full-tile singles spread across all three
    trigger queues (sync/scalar/gpsimd) in strict PE-demand order; per-DMA
    completion is ~2+us fixed latency + per-queue throughput, so finer
    splitting does not arrive sooner.  bv is broadcast on-chip (1KB row
    load + f32 PE broadcast matmul in the warmup shadow) instead of a
    128KB broadcast-write DMA.
  * ST (scores+exp) units are pure on-chip work and are interleaved as PE
    filler through every DMA-paced phase, always paired with >=430ns of
    other PE work so the scalar-engine exp (820ns/tile, ps_s bufs=2) stays
    hidden.
  * Output stores ride sync+gpsimd (never scalar: its queue is serialized
    behind the exp stream).  All AV(1) stores ride sync only - the gpsimd
    software-DGE drains ~3us at kernel end and its last DMA would gate the
    final barrier.  The last AV tile is computed as four column-group PSUM
    accumulations with muls alternating vector/scalar engines into one
    contiguous output tile (dedicated bufs, no WAR on the store rotation),
    stored with just two sync-queue triggers.
"""

import math
import os
import sys

import numpy as np

os.environ.setdefault("MYCRO_LOCAL_CACHE", "1")
if "/opt/trn_rl_repo" not in sys.path:
    sys.path.insert(0, "/opt/trn_rl_repo")

from contextlib import ExitStack

import concourse.bass as bass
import concourse.tile as tile
from concourse import bacc, mybir
from concourse.bass_utils import run_bass_kernel_spmd

B, S, IN_DIM, OUT_DIM = 16, 1024, 512, 256
THETA = 10000.0
N_CORES = 8
B_LOC = B // N_CORES          # batches per core
I_T = IN_DIM // 128           # 4 contraction tiles for the projections
O_T = OUT_DIM // 128          # 2 feature tiles
S_TILES = S // 128            # 8 sequence tiles
SC = S // 512                 # 2 sequence chunks of 512
SCALE = 1.0 / math.sqrt(IN_DIM)


F32 = mybir.dt.float32
BF16 = mybir.dt.bfloat16


def _build():
    nc = bacc.Bacc(
        "TRN2",
        target_bir_lowering=False,
        debug=False,
        enable_asserts=False,
        num_devices=N_CORES,
    )
    # x pre-tiled on host: xh[b, h, p, i, s] = x[global_batch, h*512+s, i*128+p]
    # (p-major so each DMA partition row is 1KB contiguous per i-tile)
    xh = nc.dram_tensor(
        "xh", [B_LOC, SC, 128, I_T, 512], BF16, kind="ExternalInput"
    ).ap()
    # wq/wk pre-tiled on host, o-half-major: w[p, o, i, c] = w_rot.T[i*128+p, o*128+c]
    wq = nc.dram_tensor("wq", [128, O_T, I_T, 128], BF16, kind="ExternalInput").ap()
    wk = nc.dram_tensor("wk", [128, O_T, I_T, 128], BF16, kind="ExternalInput").ap()
    wv = nc.dram_tensor("wv", [128, I_T, OUT_DIM], BF16, kind="ExternalInput").ap()
    # packed rotated q/k biases: col o = bq_r tile o, col O_T+o = bk_r tile o
    bqk = nc.dram_tensor("bqk", [128, 2 * O_T], F32, kind="ExternalInput").ap()
    bv = nc.dram_tensor("bv", [OUT_DIM], F32, kind="ExternalInput").ap()
    out = nc.dram_tensor("out", [B_LOC, S, OUT_DIM], F32, kind="ExternalOutput").ap()

    with tile.TileContext(nc) as tc, ExitStack() as ctx:
        sb = ctx.enter_context(tc.tile_pool(name="sb", bufs=1))
        small = ctx.enter_context(tc.tile_pool(name="small", bufs=4))
        ps_pj = ctx.enter_context(tc.tile_pool(name="ps_pj", bufs=3, space="PSUM"))
        ps_s = ctx.enter_context(tc.tile_pool(name="ps_s", bufs=2, space="PSUM"))
        ps_av = ctx.enter_context(tc.tile_pool(name="ps_av", bufs=3, space="PSUM"))

        S_, G_, C_, V_ = nc.sync, nc.gpsimd, nc.scalar, nc.vector

        # ---- PE warmup FIRST: dispatches right after the init barrier so
        # the clock-ramp (0.65 -> 1.2 -> 2.4 GHz over ~3-5us of continuous
        # execution) is burned while the input DMAs are in flight.
        # The warmup reads a raw, never-written scratch tensor (garbage
        # bits are fine: the PSUM result is discarded), so it has ZERO
        # dependencies and the ramp starts at kernel entry instead of
        # behind a memset.  Warmup is sized to drain just as the first
        # real operands land (~2us in), so it never delays real work.
        junk = nc.alloc_sbuf_tensor("warmjunk", [128, 512], BF16).ap()
        for n_mm in (4, 4):
            wps = ps_pj.tile([128, 512], F32, tag="pj", name="warmps")
            for _k in range(n_mm):
                nc.tensor.matmul(wps, junk[:, 0:128], junk, start=(_k == 0),
                                 stop=(_k == n_mm - 1))

        # Input loads: the head is DMA-delivery-bound (each queue sustains
        # ~55GB/s per in-flight transfer), so the critical first-group
        # operands are split in HALVES across all four trigger-capable
        # queues (sync, scalar, vector, gpsimd) and issued in just-in-time
        # demand order: each PE group's operands land right before the PE
        # reaches that group while the clock is still ramping.
        xsb = {}

        def x_tile(b, h, i):
            t = sb.tile([128, 512], BF16, name=f"x{b}h{h}i{i}")
            xsb[b, h, i] = t
            return t

        def load_xi(eng, b, h, i):
            eng.dma_start(out=x_tile(b, h, i), in_=xh[b, h, :, i, :])

        def load_xi_half(eng, b, h, i, half):
            t = xsb.get((b, h, i)) or x_tile(b, h, i)
            lo, hi = (0, 256) if half == 0 else (256, 512)
            eng.dma_start(out=t[:, lo:hi], in_=xh[b, h, :, i, lo:hi])

        wq_sb = [sb.tile([128, I_T, 128], BF16, name=f"wq{o}") for o in range(O_T)]
        wk_sb = [sb.tile([128, I_T, 128], BF16, name=f"wk{o}") for o in range(O_T)]
        wv_sb = sb.tile([128, I_T, OUT_DIM], BF16, name="wv_sb")
        bqk_sb = sb.tile([128, 2 * O_T], F32, name="bqk_sb")
        bv_bc = sb.tile([128, OUT_DIM], F32, name="bv_bc")

        def load_w_half(eng, w_dram, dst, o, half):
            # [128, I_T, 128] per-o tile split along I_T: 2 x 64KB
            lo, hi = (0, 2) if half == 0 else (2, 4)
            eng.dma_start(out=dst[:, lo:hi, :], in_=w_dram[:, o, lo:hi, :])

        def load_wv_q(eng, q):
            # wv [128, I_T, OUT_DIM] split along I_T: 4 x 64KB
            eng.dma_start(out=wv_sb[:, q, :], in_=wv[:, q, :])

        # Measured DMA behavior: per-queue slot-1 (128KB) completes
        # ~10.4-11.3; later slots ~1.3-1.4us apart per queue (delivery
        # itself ramps up).  Splitting one tensor across slots of the SAME
        # queue delays it, and sub-64KB pieces don't arrive any sooner, so
        # the schedule below uses full-tile singles spread across all three
        # trigger queues (the baseline left gpsimd idle) in strict demand
        # order.  Tiny bqk rides an early gpsimd slot (it gates the first
        # vector epilogue and thus the pj-PSUM rotation).
        # bv is broadcast on-chip (1KB row load + PE f32 broadcast matmul)
        # instead of a 128KB broadcast-write DMA that would eat ~1.3us of
        # critical-window queue capacity.
        bv_row = sb.tile([1, OUT_DIM], F32, name="bv_row")

        # Per-queue arrival ~= 8.4us + (cumulative KB ahead)/(~45-90GB/s,
        # ramping).  Demand: x00*+wq0 ~11-13, wq1 ~13.5, wk ~14-15.5,
        # wv ~16-18 (v block), x01* ~19-21.5, x1** ~22-28.
        load_xi(S_, 0, 0, 0)                           # S1: x000
        load_w_half(C_, wq, wq_sb[0], 0, 0)            # C1: wq0a
        load_w_half(G_, wq, wq_sb[0], 0, 1)            # G1: wq0b
        G_.dma_start(out=bv_row,
                     in_=bass.AP(tensor=bv.tensor, offset=bv.offset,
                                 ap=[[0, 1], bv.ap[0]]))  # G2: bv row (1KB)
        load_xi(G_, 0, 0, 1)                           # G3: x001
        load_xi(C_, 0, 0, 2)                           # C2: x002
        G_.dma_start(out=bqk_sb, in_=bqk)              # G4: bqk (4KB)
        load_w_half(S_, wq, wq_sb[1], 1, 0)            # S2: wq1a
        load_xi(G_, 0, 0, 3)                           # G5: x003
        load_w_half(C_, wq, wq_sb[1], 1, 1)            # C3: wq1b
        load_w_half(S_, wk, wk_sb[0], 0, 0)            # S3: wk0a
        load_w_half(C_, wk, wk_sb[0], 0, 1)            # C4: wk0b
        load_w_half(G_, wk, wk_sb[1], 1, 0)            # G6: wk1a
        load_w_half(G_, wk, wk_sb[1], 1, 1)            # G7: wk1b
        load_wv_q(S_, 0)                               # S4: wv_i0
        load_wv_q(C_, 1)                               # C5: wv_i1
        load_wv_q(S_, 2)                               # S5: wv_i2
        load_wv_q(G_, 3)                               # G8: wv_i3
        # h=1 x tiles (consumed from ~19us after the ST(0)/v filler block)
        load_xi(S_, 0, 1, 0)
        load_xi(C_, 0, 1, 1)
        load_xi(S_, 0, 1, 2)
        load_xi(C_, 0, 1, 3)
        # batch 1 tiles: ST filler gives these huge slack
        load_xi(C_, 1, 0, 0)
        load_xi(S_, 1, 0, 1)
        load_xi(G_, 1, 0, 2)
        load_xi(C_, 1, 0, 3)
        load_xi(G_, 1, 1, 0)
        load_xi(C_, 1, 1, 1)
        load_xi(S_, 1, 1, 2)
        load_xi(G_, 1, 1, 3)

        # broadcast bv_row across partitions on the PE (f32 ones x row) and
        # copy PSUM -> SBUF on the scalar engine; runs in the warmup shadow
        ones1 = sb.tile([1, 128], F32, name="ones1")
        nc.vector.memset(ones1, 1.0)
        ps_bv = ps_s.tile([128, OUT_DIM], F32, tag="s", name="ps_bv")
        nc.tensor.matmul(ps_bv, ones1, bv_row, start=True, stop=True)
        nc.scalar.copy(bv_bc, ps_bv)

        q_sb = [sb.tile([128, O_T, S], BF16, name=f"q{b}") for b in range(B_LOC)]
        k_sb = [sb.tile([128, O_T, S], BF16, name=f"k{b}") for b in range(B_LOC)]
        v_sb = [
            sb.tile([128, S_TILES, OUT_DIM + 1], BF16, name=f"v{b}")
            for b in range(B_LOC)
        ]
        e_sb = [sb.tile([128, S_TILES, S], BF16, name=f"e{b}") for b in range(B_LOC)]

        # softmax rowsum ones-columns: one strided memset per batch (the
        # column-0 lanes of all 8 v tiles at once) instead of 16 singles
        for b in range(B_LOC):
            nc.vector.memset(v_sb[b][:, :, 0:1], 1.0)

        # ---- work-unit generators (each unit = one PSUM group + epilogue) ----
        def qk_units(b, h):
            units = []
            for w_s, b_off, dst in (
                (wq_sb, 0, q_sb[b]),
                (wk_sb, O_T, k_sb[b]),
            ):
                for o in range(O_T):
                    def f(h=h, o=o, w_s=w_s, b_off=b_off, dst=dst):
                        ps = ps_pj.tile([128, 512], F32, tag="pj", name="pspj")
                        for i in range(I_T):
                            nc.tensor.matmul(
                                ps,
                                w_s[o][:, i, :],
                                xsb[b, h, i],
                                start=(i == 0),
                                stop=(i == I_T - 1),
                            )
                        nc.vector.tensor_scalar_add(
                            dst[:, o, h * 512:(h + 1) * 512], ps,
                            bqk_sb[:, b_off + o:b_off + o + 1],
                        )
                    units.append(f)
            return units

        def v_units(b, h):
            units = []
            for j in range(S_TILES // SC):
                def f(h=h, j=j):
                    st = h * (S_TILES // SC) + j
                    ps = ps_pj.tile([128, OUT_DIM], F32, tag="pj", name="pspj")
                    for i in range(I_T):
                        nc.tensor.matmul(
                            ps,
                            xsb[b, h, i][:, j * 128:(j + 1) * 128],
                            wv_sb[:, i, :],
                            start=(i == 0),
                            stop=(i == I_T - 1),
                        )
                    nc.vector.tensor_add(v_sb[b][:, st, 1:OUT_DIM + 1], ps, bv_bc)
                units.append(f)
            return units

        def st_units(b):
            # scores^T tile (sk on partitions, sq chunk on free) + fused exp
            units = []
            for h in range(SC):
                for sk in range(S_TILES):
                    def f(h=h, sk=sk):
                        ps = ps_s.tile([128, 512], F32, tag="s", name="pss")
                        for d in range(O_T):
                            nc.tensor.matmul(
                                ps,
                                k_sb[b][:, d, sk * 128:(sk + 1) * 128],
                                q_sb[b][:, d, h * 512:(h + 1) * 512],
                                start=(d == 0),
                                stop=(d == O_T - 1),
                            )
                        nc.scalar.activation(
                            out=e_sb[b][:, sk, h * 512:(h + 1) * 512],
                            in_=ps,
                            func=mybir.ActivationFunctionType.Exp,
                            scale=SCALE,
                        )
                    units.append(f)
            return units

        def av_units(b, split_last=False):
            units = []
            for sq in range(S_TILES):
                if split_last and sq == S_TILES - 1:
                    def f(sq=sq):
                        # four column-group PSUM accumulations: the early
                        # epilogues+stores overlap the later groups'
                        # matmuls, so only a 32-col scalar-engine mul and
                        # a 16KB store trail the final matmul
                        widths = (87, 86, 52, 32)   # A covers ones+86
                        starts = (0, 87, 173, 225)  # psum col starts
                        pss = []
                        for w, c in zip(widths, starts):
                            ps = ps_av.tile([128, w], F32, tag="av",
                                            name=f"psav{c}")
                            for sk in range(S_TILES):
                                nc.tensor.matmul(
                                    ps,
                                    e_sb[b][:, sk, sq * 128:(sq + 1) * 128],
                                    v_sb[b][:, sk, c:c + w],
                                    start=(sk == 0),
                                    stop=(sk == S_TILES - 1),
                                )
                            pss.append(ps)
                        rows = slice(sq * 128, (sq + 1) * 128)
                        # dedicated buffers (bufs=1, unique tags) so this
                        # epilogue never WAR-waits on earlier units' store
                        # DMAs draining the shared ot rotation
                        rec = small.tile([128, 1], F32, tag="reclast", bufs=1,
                                         name="rec")
                        nc.vector.reciprocal(rec, pss[0][:, 0:1])
                        # out columns: A -> 0:86, B -> 86:172, C -> 172:224,
                        # D -> 224:256.  A/C muls on vector, B/D on the
                        # scalar engine (activation Copy with per-partition
                        # scale) so consecutive group epilogues overlap.
                        # One contiguous ot tile -> only TWO store triggers
                        # (trigger exec ~0.7us is the tail bottleneck).
                        ot = small.tile([128, OUT_DIM], F32, tag="otlast",
                                        bufs=1, name="otlast")
                        nc.vector.tensor_scalar_mul(ot[:, 0:86],
                                                    pss[0][:, 1:87], rec)
                        nc.scalar.activation(
                            out=ot[:, 86:172], in_=pss[1],
                            func=mybir.ActivationFunctionType.Copy, scale=rec)
                        S_.dma_start(out=out[b, rows, 0:172], in_=ot[:, 0:172])
                        nc.vector.tensor_scalar_mul(ot[:, 172:224], pss[2], rec)
                        nc.scalar.activation(
                            out=ot[:, 224:OUT_DIM], in_=pss[3],
                            func=mybir.ActivationFunctionType.Copy, scale=rec)
                        S_.dma_start(out=out[b, rows, 172:OUT_DIM],
                                     in_=ot[:, 172:OUT_DIM])
                    units.append(f)
                    continue

                def f(sq=sq):
                    ps = ps_av.tile([128, OUT_DIM + 1], F32, tag="av", name="psav")
                    for sk in range(S_TILES):
                        nc.tensor.matmul(
                            ps,
                            e_sb[b][:, sk, sq * 128:(sq + 1) * 128],
                            v_sb[b][:, sk, :],
                            start=(sk == 0),
                            stop=(sk == S_TILES - 1),
                        )
                    rec = small.tile([128, 1], F32, tag="rec", name="rec")
                    nc.vector.reciprocal(rec, ps[:, 0:1])
                    ot = small.tile([128, OUT_DIM], F32, tag="ot", name="ot")
                    nc.vector.tensor_scalar_mul(ot, ps[:, 1:OUT_DIM + 1], rec)
                    rows = slice(sq * 128, (sq + 1) * 128)
                    # Full-tile single stores: trigger exec (~0.7us each) is
                    # the scarce resource, not BW.  AV(0) alternates the two
                    # tail-idle queues; AV(1) rides sync only — the gpsimd
                    # software-DGE drains slowly at kernel end (its last DMA
                    # would gate the final barrier by ~3us), so its last
                    # store must come early.  (Scalar's queue is backed up
                    # behind the exp stream.)
                    q = [S_, G_][sq % 2] if not split_last else S_
                    q.dma_start(out=out[b, rows, :], in_=ot)
                units.append(f)
            return units

        # ---- emission: ST units are pure on-chip work, so they are spread
        # through the DMA-paced phases as PE filler; each ST unit is paired
        # with >=1 other PE unit so the scalar-engine exp (820ns vs the ST
        # matmuls' 430ns) stays hidden behind PE work (ps_s has 2 bufs) ----
        st0 = st_units(0)
        st1 = st_units(1)
        av0 = av_units(0)
        av1 = av_units(1, split_last=True)

        for u in qk_units(0, 0):
            u()
        # ST(0, h-chunk 0, k-tiles of h=0) paired with v(0, h=0): fills the
        # wv-delivery window with on-chip score work
        for sk, (a, u) in enumerate(zip(st0[0:4], v_units(0, 0))):
            a()
            u()
        for u in qk_units(0, 1):
            u()
        for a, u in zip(st0[4:8], v_units(0, 1)):
            a()
            u()
        # batch-1 h=0 projections interleaved with ST(0, h-chunk 1); the
        # on-chip ST filler precedes each DMA-paced projection unit
        qkv10 = qk_units(1, 0) + v_units(1, 0)
        for a, u in zip(st0[8:16], qkv10):
            a()
            u()
        # batch-1 h=1 projections interleaved with first ST(1) units
        qkv11 = qk_units(1, 1) + v_units(1, 1)
        for idx, u in enumerate(qkv11):
            u()
            if idx % 2 == 1:
                st1[idx // 2]()
        # remaining 12 ST(1) units with AV(0): st,st,av,st,av x4
        si, ai = 4, 0
        for _ in range(4):
            st1[si](); si += 1
            st1[si](); si += 1
            av0[ai](); ai += 1
            st1[si](); si += 1
            av0[ai](); ai += 1
        assert si == 16 and ai == 8
        for u in av1:
            u()

    nc.compile()
    return nc


_CACHE = {}


def _get_nc():
    if "nc" not in _CACHE:
        _CACHE["nc"] = _build()
    return _CACHE["nc"]


def _rope_fold(w, bvec):
    """Fold the (position-independent) RoPE rotation into weights/bias.

    Mirrors the reference: inv_freq over arange(0, OUT_DIM, 2)/OUT_DIM,
    angle = 2*S*inv_freq, pairs (2j, 2j+1) rotated by angle_j.
    Computed in float32 to track the reference's f32 arithmetic.
    """
    exps = np.arange(0, OUT_DIM, 2, dtype=np.float32) / np.float32(OUT_DIM)
    inv = (np.float32(1.0) / np.power(np.float32(THETA), exps)).astype(np.float32)
    ang = (np.float32(2.0 * S) * inv).astype(np.float32)
    cos = np.cos(ang).astype(np.float32)
    sin = np.sin(ang).astype(np.float32)

    w2 = w.reshape(OUT_DIM // 2, 2, IN_DIM)
    wr = np.empty_like(w2)
    wr[:, 0] = cos[:, None] * w2[:, 0] - sin[:, None] * w2[:, 1]
    wr[:, 1] = sin[:, None] * w2[:, 0] + cos[:, None] * w2[:, 1]
    b2 = bvec.reshape(OUT_DIM // 2, 2)
    br = np.empty_like(b2)
    br[:, 0] = cos * b2[:, 0] - sin * b2[:, 1]
    br[:, 1] = sin * b2[:, 0] + cos * b2[:, 1]
    return wr.reshape(OUT_DIM, IN_DIM), br.reshape(OUT_DIM)


def _pack_w_halves(w_rot, bf16):
    """[OUT_DIM, IN_DIM] weight -> [128, O_T, I_T, 128] bf16 (o-half-major)."""
    wt = np.ascontiguousarray(w_rot.T)                  # [IN_DIM, OUT_DIM]
    return np.ascontiguousarray(
        wt.reshape(I_T, 128, O_T, 128).transpose(1, 2, 0, 3)
    ).astype(bf16)


def _pack_w(w, bf16):
    """[OUT_DIM, IN_DIM] weight -> [128, I_T, OUT_DIM] bf16 SBUF tile layout."""
    wt = np.ascontiguousarray(w.T)                      # [IN_DIM, OUT_DIM]
    return np.ascontiguousarray(
        wt.reshape(I_T, 128, OUT_DIM).transpose(1, 0, 2)
    ).astype(bf16)


def _prep_inputs(x, wq, bq, wk, bk, wv, bv):
    import ml_dtypes
    bf16 = ml_dtypes.bfloat16
    x = np.asarray(x, dtype=np.float32)
    wq_r, bq_r = _rope_fold(np.asarray(wq, np.float32), np.asarray(bq, np.float32))
    wk_r, bk_r = _rope_fold(np.asarray(wk, np.float32), np.asarray(bk, np.float32))
    wv = np.asarray(wv, np.float32)
    bv = np.asarray(bv, np.float32)

    bqk = np.concatenate(
        [
            np.ascontiguousarray(bq_r.reshape(O_T, 128).T),
            np.ascontiguousarray(bk_r.reshape(O_T, 128).T),
        ],
        axis=1,
    )
    shared = {
        "wq": _pack_w_halves(wq_r, bf16),
        "wk": _pack_w_halves(wk_r, bf16),
        "wv": _pack_w(wv, bf16),
        "bqk": np.ascontiguousarray(bqk),
        "bv": bv,
    }
    in_maps = []
    for c in range(N_CORES):
        shard = x[c * B_LOC:(c + 1) * B_LOC]            # [B_LOC, S, IN_DIM]
        # xh[b, h, p, i, s] = shard[b, h*512+s, i*128+p]
        xh = np.ascontiguousarray(
            shard.transpose(0, 2, 1)                     # [b, IN_DIM, S]
            .reshape(B_LOC, I_T, 128, SC, 512)
            .transpose(0, 3, 2, 1, 4)
        ).astype(bf16)
        in_maps.append({"xh": xh, **shared})
    return in_maps


def _execute(in_maps, trace=False, tmpdir=None):
    nc = _get_nc()
    return run_bass_kernel_spmd(
        nc, in_maps, core_ids=list(range(N_CORES)), trace=trace, tmpdir=tmpdir
    )


def kernel(x, wq, bq, wk, bk, wv, bv):
    in_maps = _prep_inputs(x, wq, bq, wk, bk, wv, bv)
    res = _execute(in_maps)
    return np.concatenate(
        [np.asarray(res.results[i]["out"]) for i in range(N_CORES)], axis=0
    ).astype(np.float32)



# revision 26
# speedup vs baseline: 1.0012x; 1.0012x over previous
"""Trainium2 Bass kernel: batch-sharded fused attention (nn_Attention_48893907698232).

Reference computation (per batch b):
    q = x @ wq.T + bq ; k = x @ wk.T + bk ; v = x @ wv.T + bv
    q, k <- fixed RoPE rotation (same rotation at every position)
    out  = softmax((q @ k.T) / sqrt(512)) @ v

Strategy:
  * Data-parallel over batch: 16 batches -> 2 per NeuronCore, 8 cores, no
    collectives (attention is per-batch independent).
  * The RoPE rotation is position-independent here, so it is folded into the
    projection weights/biases on the host (w_rot = R @ w, b_rot = R @ b).
  * Host pre-transposes/pre-tiles x and weights into SBUF tile layout so every
    matmul contraction dim lands on SBUF partitions with zero on-chip
    transposes and every DMA row is contiguous:
      qT[o,s]  = sum_i wqT[i,o] * xT[i,s]     (PSUM out: o on partitions)
      kT[o,s]  likewise
      v[s,o]   = sum_i xT[i,s] * wvT[i,o]     (PSUM out: s on partitions)
      ST[sk,sq]= sum_d kT[d,sk] * qT[d,sq]    -> exp(scale*ST) on ACT -> ET
      out[sq,:]= sum_sk ET[sk,sq] * [1|v]     (ones column 0 => softmax rowsum
                                               lands in output column 0)
      out  <- out[:, 1:257] * (1 / out[:, 0])
  * Softmax runs without max-subtraction: scores are ~N(0, 0.75) for this
    problem so exp() is far from overflow.
  * Matmul operands are bf16 (full-rate TensorEngine, fp32 PSUM accumulate).
  * Startup is latency-tuned: a dependency-free junk-matmul warmup block
    (reads an uninitialized scratch tensor; PSUM discarded) dispatches at
    kernel entry so the PE clock ramp (0.65->1.2->2.4GHz over ~5us of
    CONTINUOUS execution; an idle gap freezes it) burns while the input
    DMAs land.  Input loads are full-tile singles spread across all three
    trigger queues (sync/scalar/gpsimd) in strict PE-demand order; per-DMA
    completion is ~2+us fixed latency + per-queue throughput, so finer
    splitting does not arrive sooner.  bv is broadcast on-chip (1KB row
    load + f32 PE broadcast matmul in the warmup shadow) instead of a
    128KB broadcast-write DMA.
  * ST (scores+exp) units are pure on-chip work and are interleaved as PE
    filler through every DMA-paced phase, always paired with >=430ns of
    other PE work so the scalar-engine exp (820ns/tile, ps_s bufs=2) stays
    hidden.
  * Output stores ride sync+gpsimd (never scalar: its queue is serialized
    behind the exp stream).  All AV(1) stores ride sync only - the gpsimd
    software-DGE drains ~3us at kernel end and its last DMA would gate the
    final barrier.  The last AV tile is computed as four column-group PSUM
    accumulations with muls alternating vector/scalar engines into one
    contiguous output tile (dedicated bufs, no WAR on the store rotation),
    stored with just two sync-queue triggers.
"""

import math
import os
import sys

import numpy as np

os.environ.setdefault("MYCRO_LOCAL_CACHE", "1")
if "/opt/trn_rl_repo" not in sys.path:
    sys.path.insert(0, "/opt/trn_rl_repo")

from contextlib import ExitStack

import concourse.bass as bass
import concourse.tile as tile
from concourse import bacc, mybir
from concourse.bass_utils import run_bass_kernel_spmd

B, S, IN_DIM, OUT_DIM = 16, 1024, 512, 256
THETA = 10000.0
N_CORES = 8
B_LOC = B // N_CORES          # batches per core
I_T = IN_DIM // 128           # 4 contraction tiles for the projections
O_T = OUT_DIM // 128          # 2 feature tiles
S_TILES = S // 128            # 8 sequence tiles
SC = S // 512                 # 2 sequence chunks of 512
SCALE = 1.0 / math.sqrt(IN_DIM)


F32 = mybir.dt.float32
BF16 = mybir.dt.bfloat16


def _build():
    nc = bacc.Bacc(
        "TRN2",
        target_bir_lowering=False,
        debug=False,
        enable_asserts=False,
        num_devices=N_CORES,
    )
    # x pre-tiled on host: xh[b, h, p, i, s] = x[global_batch, h*512+s, i*128+p]
    # (p-major so each DMA partition row is 1KB contiguous per i-tile)
    xh = nc.dram_tensor(
        "xh", [B_LOC, SC, 128, I_T, 512], BF16, kind="ExternalInput"
    ).ap()
    # wq/wk pre-tiled on host, o-half-major: w[p, o, i, c] = w_rot.T[i*128+p, o*128+c]
    wq = nc.dram_tensor("wq", [128, O_T, I_T, 128], BF16, kind="ExternalInput").ap()
    wk = nc.dram_tensor("wk", [128, O_T, I_T, 128], BF16, kind="ExternalInput").ap()
    wv = nc.dram_tensor("wv", [128, I_T, OUT_DIM], BF16, kind="ExternalInput").ap()
    # packed rotated q/k biases: col o = bq_r tile o, col O_T+o = bk_r tile o
    bqk = nc.dram_tensor("bqk", [128, 2 * O_T], F32, kind="ExternalInput").ap()
    bv = nc.dram_tensor("bv", [OUT_DIM], F32, kind="ExternalInput").ap()
    out = nc.dram_tensor("out", [B_LOC, S, OUT_DIM], F32, kind="ExternalOutput").ap()

    with tile.TileContext(nc) as tc, ExitStack() as ctx:
        sb = ctx.enter_context(tc.tile_pool(name="sb", bufs=1))
        small = ctx.enter_context(tc.tile_pool(name="small", bufs=4))
        ps_pj = ctx.enter_context(tc.tile_pool(name="ps_pj", bufs=3, space="PSUM"))
        ps_s = ctx.enter_context(tc.tile_pool(name="ps_s", bufs=2, space="PSUM"))
        ps_av = ctx.enter_context(tc.tile_pool(name="ps_av", bufs=3, space="PSUM"))

        S_, G_, C_, V_ = nc.sync, nc.gpsimd, nc.scalar, nc.vector

        # ---- PE warmup FIRST: dispatches right after the init barrier so
        # the clock-ramp (0.65 -> 1.2 -> 2.4 GHz over ~3-5us of continuous
        # execution) is burned while the input DMAs are in flight.
        # The warmup reads a raw, never-written scratch tensor (garbage
        # bits are fine: the PSUM result is discarded), so it has ZERO
        # dependencies and the ramp starts at kernel entry instead of
        # behind a memset.  Warmup is sized to drain just as the first
        # real operands land (~2us in), so it never delays real work.
        junk = nc.alloc_sbuf_tensor("warmjunk", [128, 512], BF16).ap()
        for n_mm in (4, 4, 1):
            wps = ps_pj.tile([128, 512], F32, tag="pj", name="warmps")
            for _k in range(n_mm):
                nc.tensor.matmul(wps, junk[:, 0:128], junk, start=(_k == 0),
                                 stop=(_k == n_mm - 1))

        # Input loads: the head is DMA-delivery-bound (each queue sustains
        # ~55GB/s per in-flight transfer), so the critical first-group
        # operands are split in HALVES across all four trigger-capable
        # queues (sync, scalar, vector, gpsimd) and issued in just-in-time
        # demand order: each PE group's operands land right before the PE
        # reaches that group while the clock is still ramping.
        xsb = {}

        def x_tile(b, h, i):
            t = sb.tile([128, 512], BF16, name=f"x{b}h{h}i{i}")
            xsb[b, h, i] = t
            return t

        def load_xi(eng, b, h, i):
            eng.dma_start(out=x_tile(b, h, i), in_=xh[b, h, :, i, :])

        def load_xi_half(eng, b, h, i, half):
            t = xsb.get((b, h, i)) or x_tile(b, h, i)
            lo, hi = (0, 256) if half == 0 else (256, 512)
            eng.dma_start(out=t[:, lo:hi], in_=xh[b, h, :, i, lo:hi])

        wq_sb = [sb.tile([128, I_T, 128], BF16, name=f"wq{o}") for o in range(O_T)]
        wk_sb = [sb.tile([128, I_T, 128], BF16, name=f"wk{o}") for o in range(O_T)]
        wv_sb = sb.tile([128, I_T, OUT_DIM], BF16, name="wv_sb")
        bqk_sb = sb.tile([128, 2 * O_T], F32, name="bqk_sb")
        bv_bc = sb.tile([128, OUT_DIM], F32, name="bv_bc")

        def load_w_half(eng, w_dram, dst, o, half):
            # [128, I_T, 128] per-o tile split along I_T: 2 x 64KB
            lo, hi = (0, 2) if half == 0 else (2, 4)
            eng.dma_start(out=dst[:, lo:hi, :], in_=w_dram[:, o, lo:hi, :])

        def load_wv_q(eng, q):
            # wv [128, I_T, OUT_DIM] split along I_T: 4 x 64KB
            eng.dma_start(out=wv_sb[:, q, :], in_=wv[:, q, :])

        # Measured DMA behavior: per-queue slot-1 (128KB) completes
        # ~10.4-11.3; later slots ~1.3-1.4us apart per queue (delivery
        # itself ramps up).  Splitting one tensor across slots of the SAME
        # queue delays it, and sub-64KB pieces don't arrive any sooner, so
        # the schedule below uses full-tile singles spread across all three
        # trigger queues (the baseline left gpsimd idle) in strict demand
        # order.  Tiny bqk rides an early gpsimd slot (it gates the first
        # vector epilogue and thus the pj-PSUM rotation).
        # bv is broadcast on-chip (1KB row load + PE f32 broadcast matmul)
        # instead of a 128KB broadcast-write DMA that would eat ~1.3us of
        # critical-window queue capacity.
        bv_row = sb.tile([1, OUT_DIM], F32, name="bv_row")

        # Per-queue arrival ~= 8.4us + (cumulative KB ahead)/(~45-90GB/s,
        # ramping).  Demand: x00*+wq0 ~11-13, wq1 ~13.5, wk ~14-15.5,
        # wv ~16-18 (v block), x01* ~19-21.5, x1** ~22-28.
        load_xi(S_, 0, 0, 0)                           # S1: x000
        load_w_half(C_, wq, wq_sb[0], 0, 0)            # C1: wq0a
        load_w_half(G_, wq, wq_sb[0], 0, 1)            # G1: wq0b
        G_.dma_start(out=bv_row,
                     in_=bass.AP(tensor=bv.tensor, offset=bv.offset,
                                 ap=[[0, 1], bv.ap[0]]))  # G2: bv row (1KB)
        load_xi(G_, 0, 0, 1)                           # G3: x001
        load_xi(C_, 0, 0, 2)                           # C2: x002
        G_.dma_start(out=bqk_sb, in_=bqk)              # G4: bqk (4KB)
        load_w_half(S_, wq, wq_sb[1], 1, 0)            # S2: wq1a
        load_xi(G_, 0, 0, 3)                           # G5: x003
        load_w_half(C_, wq, wq_sb[1], 1, 1)            # C3: wq1b
        load_w_half(S_, wk, wk_sb[0], 0, 0)            # S3: wk0a
        load_w_half(C_, wk, wk_sb[0], 0, 1)            # C4: wk0b
        load_w_half(G_, wk, wk_sb[1], 1, 0)            # G6: wk1a
        load_w_half(G_, wk, wk_sb[1], 1, 1)            # G7: wk1b
        load_wv_q(S_, 0)                               # S4: wv_i0
        load_wv_q(C_, 1)                               # C5: wv_i1
        load_wv_q(S_, 2)                               # S5: wv_i2
        load_wv_q(G_, 3)                               # G8: wv_i3
        # h=1 x tiles (consumed from ~19us after the ST(0)/v filler block)
        load_xi(S_, 0, 1, 0)
        load_xi(C_, 0, 1, 1)
        load_xi(S_, 0, 1, 2)
        load_xi(C_, 0, 1, 3)
        # batch 1 tiles: ST filler gives these huge slack
        load_xi(C_, 1, 0, 0)
        load_xi(S_, 1, 0, 1)
        load_xi(G_, 1, 0, 2)
        load_xi(C_, 1, 0, 3)
        load_xi(G_, 1, 1, 0)
        load_xi(C_, 1, 1, 1)
        load_xi(S_, 1, 1, 2)
        load_xi(G_, 1, 1, 3)

        # broadcast bv_row across partitions on the PE (f32 ones x row) and
        # copy PSUM -> SBUF on the scalar engine; runs in the warmup shadow
        ones1 = sb.tile([1, 128], F32, name="ones1")
        nc.vector.memset(ones1, 1.0)
        ps_bv = ps_s.tile([128, OUT_DIM], F32, tag="s", name="ps_bv")
        nc.tensor.matmul(ps_bv, ones1, bv_row, start=True, stop=True)
        nc.scalar.copy(bv_bc, ps_bv)

        q_sb = [sb.tile([128, O_T, S], BF16, name=f"q{b}") for b in range(B_LOC)]
        k_sb = [sb.tile([128, O_T, S], BF16, name=f"k{b}") for b in range(B_LOC)]
        v_sb = [
            sb.tile([128, S_TILES, OUT_DIM + 1], BF16, name=f"v{b}")
            for b in range(B_LOC)
        ]
        e_sb = [sb.tile([128, S_TILES, S], BF16, name=f"e{b}") for b in range(B_LOC)]

        # softmax rowsum ones-columns: one strided memset per batch (the
        # column-0 lanes of all 8 v tiles at once) instead of 16 singles
        for b in range(B_LOC):
            nc.vector.memset(v_sb[b][:, :, 0:1], 1.0)

        # ---- work-unit generators (each unit = one PSUM group + epilogue) ----
        def qk_units(b, h):
            units = []
            for w_s, b_off, dst in (
                (wq_sb, 0, q_sb[b]),
                (wk_sb, O_T, k_sb[b]),
            ):
                for o in range(O_T):
                    def f(h=h, o=o, w_s=w_s, b_off=b_off, dst=dst):
                        ps = ps_pj.tile([128, 512], F32, tag="pj", name="pspj")
                        for i in range(I_T):
                            nc.tensor.matmul(
                                ps,
                                w_s[o][:, i, :],
                                xsb[b, h, i],
                                start=(i == 0),
                                stop=(i == I_T - 1),
                            )
                        nc.vector.tensor_scalar_add(
                            dst[:, o, h * 512:(h + 1) * 512], ps,
                            bqk_sb[:, b_off + o:b_off + o + 1],
                        )
                    units.append(f)
            return units

        def v_units(b, h):
            units = []
            for j in range(S_TILES // SC):
                def f(h=h, j=j):
                    st = h * (S_TILES // SC) + j
                    ps = ps_pj.tile([128, OUT_DIM], F32, tag="pj", name="pspj")
                    for i in range(I_T):
                        nc.tensor.matmul(
                            ps,
                            xsb[b, h, i][:, j * 128:(j + 1) * 128],
                            wv_sb[:, i, :],
                            start=(i == 0),
                            stop=(i == I_T - 1),
                        )
                    nc.vector.tensor_add(v_sb[b][:, st, 1:OUT_DIM + 1], ps, bv_bc)
                units.append(f)
            return units

        def st_units(b):
            # scores^T tile (sk on partitions, sq chunk on free) + fused exp
            units = []
            for h in range(SC):
                for sk in range(S_TILES):
                    def f(h=h, sk=sk):
                        ps = ps_s.tile([128, 512], F32, tag="s", name="pss")
                        for d in range(O_T):
                            nc.tensor.matmul(
                                ps,
                                k_sb[b][:, d, sk * 128:(sk + 1) * 128],
                                q_sb[b][:, d, h * 512:(h + 1) * 512],
                                start=(d == 0),
                                stop=(d == O_T - 1),
                            )
                        nc.scalar.activation(
                            out=e_sb[b][:, sk, h * 512:(h + 1) * 512],
                            in_=ps,
                            func=mybir.ActivationFunctionType.Exp,
                            scale=SCALE,
                        )
                    units.append(f)
            return units

        def av_units(b, split_last=False):
            units = []
            for sq in range(S_TILES):
                if split_last and sq == S_TILES - 1:
                    def f(sq=sq):
                        # four column-group PSUM accumulations: the early
                        # epilogues+stores overlap the later groups'
                        # matmuls, so only a 32-col scalar-engine mul and
                        # a 16KB store trail the final matmul
                        widths = (87, 86, 52, 32)   # A covers ones+86
                        starts = (0, 87, 173, 225)  # psum col starts
                        pss = []
                        for w, c in zip(widths, starts):
                            ps = ps_av.tile([128, w], F32, tag="av",
                                            name=f"psav{c}")
                            for sk in range(S_TILES):
                                nc.tensor.matmul(
                                    ps,
                                    e_sb[b][:, sk, sq * 128:(sq + 1) * 128],
                                    v_sb[b][:, sk, c:c + w],
                                    start=(sk == 0),
                                    stop=(sk == S_TILES - 1),
                                )
                            pss.append(ps)
                        rows = slice(sq * 128, (sq + 1) * 128)
                        # dedicated buffers (bufs=1, unique tags) so this
                        # epilogue never WAR-waits on earlier units' store
                        # DMAs draining the shared ot rotation
                        rec = small.tile([128, 1], F32, tag="reclast", bufs=1,
                                         name="rec")
                        nc.vector.reciprocal(rec, pss[0][:, 0:1])
                        # out columns: A -> 0:86, B -> 86:172, C -> 172:224,
                        # D -> 224:256.  A/C muls on vector, B/D on the
                        # scalar engine (activation Copy with per-partition
                        # scale) so consecutive group epilogues overlap.
                        # One contiguous ot tile -> only TWO store triggers
                        # (trigger exec ~0.7us is the tail bottleneck).
                        ot = small.tile([128, OUT_DIM], F32, tag="otlast",
                                        bufs=1, name="otlast")
                        nc.vector.tensor_scalar_mul(ot[:, 0:86],
                                                    pss[0][:, 1:87], rec)
                        nc.scalar.activation(
                            out=ot[:, 86:172], in_=pss[1],
                            func=mybir.ActivationFunctionType.Copy, scale=rec)
                        S_.dma_start(out=out[b, rows, 0:172], in_=ot[:, 0:172])
                        nc.vector.tensor_scalar_mul(ot[:, 172:224], pss[2], rec)
                        nc.scalar.activation(
                            out=ot[:, 224:OUT_DIM], in_=pss[3],
                            func=mybir.ActivationFunctionType.Copy, scale=rec)
                        S_.dma_start(out=out[b, rows, 172:OUT_DIM],
                                     in_=ot[:, 172:OUT_DIM])
                    units.append(f)
                    continue

                def f(sq=sq):
                    ps = ps_av.tile([128, OUT_DIM + 1], F32, tag="av", name="psav")
                    for sk in range(S_TILES):
                        nc.tensor.matmul(
                            ps,
                            e_sb[b][:, sk, sq * 128:(sq + 1) * 128],
                            v_sb[b][:, sk, :],
                            start=(sk == 0),
                            stop=(sk == S_TILES - 1),
                        )
                    rec = small.tile([128, 1], F32, tag="rec", name="rec")
                    nc.vector.reciprocal(rec, ps[:, 0:1])
                    ot = small.tile([128, OUT_DIM], F32, tag="ot", name="ot")
                    nc.vector.tensor_scalar_mul(ot, ps[:, 1:OUT_DIM + 1], rec)
                    rows = slice(sq * 128, (sq + 1) * 128)
                    # Full-tile single stores: trigger exec (~0.7us each) is
                    # the scarce resource, not BW.  AV(0) alternates the two
                    # tail-idle queues; AV(1) rides sync only — the gpsimd
                    # software-DGE drains slowly at kernel end (its last DMA
                    # would gate the final barrier by ~3us), so its last
                    # store must come early.  (Scalar's queue is backed up
                    # behind the exp stream.)
                    q = [S_, G_][sq % 2] if not split_last else S_
                    q.dma_start(out=out[b, rows, :], in_=ot)
                units.append(f)
            return units

        # ---- emission: ST units are pure on-chip work, so they are spread
        # through the DMA-paced phases as PE filler; each ST unit is paired
        # with >=1 other PE unit so the scalar-engine exp (820ns vs the ST
        # matmuls' 430ns) stays hidden behind PE work (ps_s has 2 bufs) ----
        st0 = st_units(0)
        st1 = st_units(1)
        av0 = av_units(0)
        av1 = av_units(1, split_last=True)

        for u in qk_units(0, 0):
            u()
        # ST(0, h-chunk 0, k-tiles of h=0) paired with v(0, h=0): fills the
        # wv-delivery window with on-chip score work
        for sk, (a, u) in enumerate(zip(st0[0:4], v_units(0, 0))):
            a()
            u()
        for u in qk_units(0, 1):
            u()
        for a, u in zip(st0[4:8], v_units(0, 1)):
            a()
            u()
        # batch-1 h=0 projections interleaved with ST(0, h-chunk 1); the
        # on-chip ST filler precedes each DMA-paced projection unit
        qkv10 = qk_units(1, 0) + v_units(1, 0)
        for a, u in zip(st0[8:16], qkv10):
            a()
            u()
        # batch-1 h=1 projections interleaved with first ST(1) units
        qkv11 = qk_units(1, 1) + v_units(1, 1)
        for idx, u in enumerate(qkv11):
            u()
            if idx % 2 == 1:
                st1[idx // 2]()
        # remaining 12 ST(1) units with AV(0): st,st,av,st,av x4
        si, ai = 4, 0
        for _ in range(4):
            st1[si](); si += 1
            st1[si](); si += 1
            av0[ai](); ai += 1
            st1[si](); si += 1
            av0[ai](); ai += 1
        assert si == 16 and ai == 8
        for u in av1:
            u()

    nc.compile()
    return nc


_CACHE = {}


def _get_nc():
    if "nc" not in _CACHE:
        _CACHE["nc"] = _build()
    return _CACHE["nc"]


def _rope_fold(w, bvec):
    """Fold the (position-independent) RoPE rotation into weights/bias.

    Mirrors the reference: inv_freq over arange(0, OUT_DIM, 2)/OUT_DIM,
    angle = 2*S*inv_freq, pairs (2j, 2j+1) rotated by angle_j.
    Computed in float32 to track the reference's f32 arithmetic.
    """
    exps = np.arange(0, OUT_DIM, 2, dtype=np.float32) / np.float32(OUT_DIM)
    inv = (np.float32(1.0) / np.power(np.float32(THETA), exps)).astype(np.float32)
    ang = (np.float32(2.0 * S) * inv).astype(np.float32)
    cos = np.cos(ang).astype(np.float32)
    sin = np.sin(ang).astype(np.float32)

    w2 = w.reshape(OUT_DIM // 2, 2, IN_DIM)
    wr = np.empty_like(w2)
    wr[:, 0] = cos[:, None] * w2[:, 0] - sin[:, None] * w2[:, 1]
    wr[:, 1] = sin[:, None] * w2[:, 0] + cos[:, None] * w2[:, 1]
    b2 = bvec.reshape(OUT_DIM // 2, 2)
    br = np.empty_like(b2)
    br[:, 0] = cos * b2[:, 0] - sin * b2[:, 1]
    br[:, 1] = sin * b2[:, 0] + cos * b2[:, 1]
    return wr.reshape(OUT_DIM, IN_DIM), br.reshape(OUT_DIM)


def _pack_w_halves(w_rot, bf16):
    """[OUT_DIM, IN_DIM] weight -> [128, O_T, I_T, 128] bf16 (o-half-major)."""
    wt = np.ascontiguousarray(w_rot.T)                  # [IN_DIM, OUT_DIM]
    return np.ascontiguousarray(
        wt.reshape(I_T, 128, O_T, 128).transpose(1, 2, 0, 3)
    ).astype(bf16)


def _pack_w(w, bf16):
    """[OUT_DIM, IN_DIM] weight -> [128, I_T, OUT_DIM] bf16 SBUF tile layout."""
    wt = np.ascontiguousarray(w.T)                      # [IN_DIM, OUT_DIM]
    return np.ascontiguousarray(
        wt.reshape(I_T, 128, OUT_DIM).transpose(1, 0, 2)
    ).astype(bf16)


def _prep_inputs(x, wq, bq, wk, bk, wv, bv):
    import ml_dtypes
    bf16 = ml_dtypes.bfloat16
    x = np.asarray(x, dtype=np.float32)
    wq_r, bq_r = _rope_fold(np.asarray(wq, np.float32), np.asarray(bq, np.float32))
    wk_r, bk_r = _rope_fold(np.asarray(wk, np.float32), np.asarray(bk, np.float32))
    wv = np.asarray(wv, np.float32)
    bv = np.asarray(bv, np.float32)

    bqk = np.concatenate(
        [
            np.ascontiguousarray(bq_r.reshape(O_T, 128).T),
            np.ascontiguousarray(bk_r.reshape(O_T, 128).T),
        ],
        axis=1,
    )
    shared = {
        "wq": _pack_w_halves(wq_r, bf16),
        "wk": _pack_w_halves(wk_r, bf16),
        "wv": _pack_w(wv, bf16),
        "bqk": np.ascontiguousarray(bqk),
        "bv": bv,
    }
    in_maps = []
    for c in range(N_CORES):
        shard = x[c * B_LOC:(c + 1) * B_LOC]            # [B_LOC, S, IN_DIM]
        # xh[b, h, p, i, s] = shard[b, h*512+s, i*128+p]
        xh = np.ascontiguousarray(
            shard.transpose(0, 2, 1)                     # [b, IN_DIM, S]
            .reshape(B_LOC, I_T, 128, SC, 512)
            .transpose(0, 3, 2, 1, 4)
        ).astype(bf16)
        in_maps.append({"xh": xh, **shared})
    return in_maps


def _execute(in_maps, trace=False, tmpdir=None):
    nc = _get_nc()
    return run_bass_kernel_spmd(
        nc, in_maps, core_ids=list(range(N_CORES)), trace=trace, tmpdir=tmpdir
    )


def kernel(x, wq, bq, wk, bk, wv, bv):
    in_maps = _prep_inputs(x, wq, bq, wk, bk, wv, bv)
    res = _execute(in_maps)
    return np.concatenate(
        [np.asarray(res.results[i]["out"]) for i in range(N_CORES)], axis=0
    ).astype(np.float32)

